# revision 1
# baseline (speedup 1.0000x reference)
"""GroupedAttention Trainium2 kernel (8 NeuronCores, SPMD, no collectives).

Problem: x[2,2048,1024] -> grouped qkv (G=8 block-diag) -> 16-head attention
-> grouped proj (G=8 block-diag) + bias.

Sharding: core c owns heads (2c, 2c+1) and proj group c. The proj group c
consumes exactly the attention outputs of heads 2c/2c+1 and produces output
channels [128c, 128c+128) -- so each core computes an independent channel
slice of the final output; outputs are concatenated on the host.

The qkv grouping does NOT align with heads (each qkv group emits a mixed
384-channel slice), so per core we hand it the three 128-channel x-slices
(for its q, k and v blocks) pre-transposed to channel-major [128, B*N],
plus the matching [128(in),128(out)] weight blocks.
"""

import numpy as np
from contextlib import ExitStack

import concourse.bass as bass
import concourse.tile as tile
from concourse import bacc, mybir
from concourse.bass_utils import run_bass_kernel_spmd

F32 = mybir.dt.float32
F32R = mybir.dt.float32r
EXP = mybir.ActivationFunctionType.Exp

B = 2
N = 2048
C = 1024
H = 16
G = 8
D = 64          # head dim
BN = B * N      # 4096
W = 512         # attention n-window per round
NB = N // W     # rounds per batch = 4
MT = N // 128   # m-tiles per batch = 16
SCALE = D ** -0.5

_CACHE = {}


def _r(ap):
    return ap if ap.dtype == F32R else ap.bitcast(F32R)


def _build_nc():
    nc = bacc.Bacc("TRN2", target_bir_lowering=False, debug=False, num_devices=8)

    xq = nc.dram_tensor("xq", [128, BN], F32, kind="ExternalInput").ap()
    xk = nc.dram_tensor("xk", [128, BN], F32, kind="ExternalInput").ap()
    xv = nc.dram_tensor("xv", [128, BN], F32, kind="ExternalInput").ap()
    wq = nc.dram_tensor("wq", [128, 128], F32, kind="ExternalInput").ap()
    wk = nc.dram_tensor("wk", [128, 128], F32, kind="ExternalInput").ap()
    wv = nc.dram_tensor("wv", [128, 256], F32, kind="ExternalInput").ap()
    wp0 = nc.dram_tensor("wp0", [64, 256], F32, kind="ExternalInput").ap()
    wp1 = nc.dram_tensor("wp1", [64, 256], F32, kind="ExternalInput").ap()
    bias = nc.dram_tensor("bias", [128, 128], F32, kind="ExternalInput").ap()
    y = nc.dram_tensor("y", [B, N, 128], F32, kind="ExternalOutput").ap()

    with ExitStack() as ctx:
        tc = ctx.enter_context(tile.TileContext(nc))
        nc_ = tc.nc

        persist = ctx.enter_context(tc.tile_pool(name="persist", bufs=1))

        # ---- load weights / constants ----
        wq_t = persist.tile([128, 128], F32R, tag="wq")
        nc_.gpsimd.dma_start(out=wq_t, in_=wq)
        wk_t = persist.tile([128, 128], F32R, tag="wk")
        nc_.gpsimd.dma_start(out=wk_t, in_=wk)
        wv_t = persist.tile([128, 256], F32R, tag="wv")
        nc_.gpsimd.dma_start(out=wv_t, in_=wv)
        wp0_t = persist.tile([64, 256], F32R, tag="wp0")
        nc_.gpsimd.dma_start(out=wp0_t, in_=wp0)
        wp1_t = persist.tile([64, 256], F32R, tag="wp1")
        nc_.gpsimd.dma_start(out=wp1_t, in_=wp1)
        bias_t = persist.tile([128, 128], F32, tag="bias")
        nc_.gpsimd.dma_start(out=bias_t, in_=bias)

        # ---- load x slices (channel-major) ----
        xq_t = persist.tile([128, BN], F32R, tag="xq")
        xk_t = persist.tile([128, BN], F32R, tag="xk")
        xv_t = persist.tile([128, BN], F32R, tag="xv")
        for i in range(4):
            s = slice(i * 1024, (i + 1) * 1024)
            nc_.gpsimd.dma_start(out=xq_t[:, s], in_=xq[:, s])
            nc_.gpsimd.dma_start(out=xk_t[:, s], in_=xk[:, s])
            nc_.gpsimd.dma_start(out=xv_t[:, s], in_=xv[:, s])

        # persistent activations
        qT = [persist.tile([128, N], F32R, tag=f"qT{b}", name=f"qT{b}")
              for b in range(B)]   # rows 0:64 h0, 64:128 h1
        kT = [persist.tile([128, N], F32R, tag=f"kT{b}", name=f"kT{b}")
              for b in range(B)]
        # v_aug[b*2+h]: [128(m), MT, 65] ; col 64 = ones (softmax denominator)
        vaug = [persist.tile([128, MT, 65], F32R, tag=f"vaug{i}", name=f"vaug{i}")
                for i in range(4)]
        ones_f = persist.tile([128, MT, 1], F32, tag="ones_f")
        nc_.gpsimd.memset(ones_f, 1.0)
        for t in vaug:
            nc_.vector.tensor_copy(out=t[:, :, 64:65], in_=ones_f)
        # ones row at partition 64, used to broadcast the softmax denominator
        # (which lands at partition 64 of the AV psum) across 64 partitions.
        ones65 = persist.tile([65, 64], F32R, tag="ones65")
        ones65_f = persist.tile([65, 64], F32, tag="ones65_f")
        nc_.gpsimd.memset(ones65_f[64:65, :], 1.0)
        nc_.vector.tensor_copy(out=ones65[64:65, :], in_=ones65_f[64:65, :])
        # normalized, stacked attention outputs per (b, h): [64(d), N]
        stk = [[persist.tile([64, N], F32R, tag=f"stk{b}{h}", name=f"stk{b}{h}")
                for h in range(2)]
               for b in range(B)]

        # ---- phase 1: qT / kT / v ----
        with tc.tile_pool(name="ph1", bufs=3, space="PSUM") as ph1:
            for i in range(8):
                s = slice(i * 512, (i + 1) * 512)
                b_, s_ = divmod(i * 512, N)
                sl = slice(s_, s_ + 512)
                pq = ph1.tile([128, 512], F32, tag="qk")
                nc_.tensor.matmul(pq, _r(wq_t), _r(xq_t[:, s]), start=True, stop=True)
                nc_.scalar.activation(out=qT[b_][:, sl], in_=pq, func=mybir.ActivationFunctionType.Copy)
                pk = ph1.tile([128, 512], F32, tag="qk")
                nc_.tensor.matmul(pk, _r(wk_t), _r(xk_t[:, s]), start=True, stop=True)
                nc_.scalar.activation(out=kT[b_][:, sl], in_=pk, func=mybir.ActivationFunctionType.Copy)
            for g in range(B * MT):
                b, mt = divmod(g, MT)
                pv = ph1.tile([128, 256], F32, tag="v")
                nc_.tensor.matmul(
                    pv, _r(xv_t[:, g * 128:(g + 1) * 128]), _r(wv_t),
                    start=True, stop=True,
                )
                nc_.vector.tensor_copy(out=vaug[b * 2][:, mt, 0:64], in_=pv[:, 0:64])
                nc_.scalar.activation(out=vaug[b * 2 + 1][:, mt, 0:64], in_=pv[:, 64:128], func=mybir.ActivationFunctionType.Copy)

        # ---- phase 2: attention ----
        with tc.tile_pool(name="stp", bufs=2, space="PSUM") as stp, \
             tc.tile_pool(name="avp", bufs=4, space="PSUM") as avp, \
             tc.tile_pool(name="ptp", bufs=4) as ptp, \
             tc.tile_pool(name="nrm", bufs=4) as nrm, \
             tc.tile_pool(name="outp", bufs=4) as outp:

            for b in range(B):
                for nb in range(NB):
                    n0 = nb * W
                    av = [avp.tile([128, W], F32, tag="av", name=f"av{b}{nb}{i}")
                          for i in range(2)]

                    def emit_av(mt, pt):
                        for h in range(2):
                            nc_.tensor.matmul(
                                av[h][0:65, :],
                                _r(vaug[b * 2 + h][:, mt, :]),
                                _r(pt[:, h * W:(h + 1) * W]),
                                start=(mt == 0), stop=(mt == MT - 1),
                            )

                    # software pipeline: PE does scores(mt) then AV(mt-1)
                    # while ACT runs exp(mt); AV(mt) only needs pt(mt).
                    prev = None
                    for mt in range(MT):
                        m0 = mt * 128
                        st = stp.tile([128, 2 * W], F32, tag="st")
                        for h in range(2):
                            hs = slice(h * 64, (h + 1) * 64)
                            nc_.tensor.matmul(
                                st[:, h * W:(h + 1) * W],
                                _r(kT[b][hs, m0:m0 + 128]),
                                _r(qT[b][hs, n0:n0 + W]),
                                start=True, stop=True,
                            )
                        if prev is not None:
                            emit_av(*prev)
                        pt = ptp.tile([128, 2 * W], F32R, tag="pt")
                        nc_.scalar.activation(out=pt, in_=st, func=EXP, scale=SCALE)
                        prev = (mt, pt)
                    emit_av(*prev)
                    # normalize: rows 0:64 of av are unnormalized out^T,
                    # row 64 is the softmax denominator Z[n].
                    for h in range(2):
                        zr = nrm.tile([65, W], F32R, tag="zr")
                        nc_.vector.tensor_copy(out=zr[64:65, :], in_=av[h][64:65, :])
                        bz = stp.tile([64, W], F32, tag="st", name=f"bz{b}{nb}{h}")
                        nc_.tensor.matmul(
                            bz, _r(ones65[64:65, :]), _r(zr[64:65, :]),
                            start=True, stop=True,
                        )
                        rbz = nrm.tile([64, W], F32, tag="rbz")
                        nc_.vector.reciprocal_approx_fast(out=rbz, in_=bz)
                        nc_.vector.tensor_mul(
                            stk[b][h][:, nb * W:(nb + 1) * W],
                            av[h][0:64, :],
                            rbz,
                        )

                # ---- phase 3: proj for batch b ----
                for nt in range(MT):
                    s = slice(nt * 128, (nt + 1) * 128)
                    pp = avp.tile([128, 256], F32, tag="av")
                    nc_.tensor.matmul(pp, _r(stk[b][0][:, s]), _r(wp0_t),
                                      start=True, stop=False)
                    nc_.tensor.matmul(pp, _r(stk[b][1][:, s]), _r(wp1_t),
                                      start=False, stop=True)
                    ot = outp.tile([128, 128], F32, tag="ot")
                    nc_.vector.tensor_add(ot, pp[:, 0:128], bias_t)
                    nc_.gpsimd.dma_start(out=y[b, s, :], in_=ot)

    nc.finalize()
    return nc


def _core_inputs(x, w_qkv, w_proj, b_proj, c):
    h0 = 2 * c
    gq, oq = divmod(64 * h0, 384)
    gk, ok = divmod(C + 64 * h0, 384)
    gv, ov = divmod(2 * C + 64 * h0, 384)

    def xsl(g):
        # [B,N,128] slice -> channel-major [128, B*N]
        return np.ascontiguousarray(
            x[:, :, 128 * g:128 * (g + 1)].reshape(BN, 128).T
        )

    wv = np.zeros((128, 256), np.float32)
    wv[:, 0:128] = w_qkv[gv][:, ov:ov + 128]
    wp = w_proj[c]
    wp0 = np.zeros((64, 256), np.float32)
    wp0[:, 0:128] = wp[0:64, :]
    wp1 = np.zeros((64, 256), np.float32)
    wp1[:, 0:128] = wp[64:128, :]
    return {
        "xq": xsl(gq),
        "xk": xsl(gk),
        "xv": xsl(gv),
        "wq": np.ascontiguousarray(w_qkv[gq][:, oq:oq + 128]),
        "wk": np.ascontiguousarray(w_qkv[gk][:, ok:ok + 128]),
        "wv": wv,
        "wp0": wp0,
        "wp1": wp1,
        "bias": np.ascontiguousarray(
            np.broadcast_to(b_proj[128 * c:128 * (c + 1)], (128, 128))
        ).astype(np.float32),
    }


def kernel(x, w_qkv, w_proj, b_proj, _trace=False, _trace_kwargs=None):
    x = np.asarray(x, np.float32)
    w_qkv = np.asarray(w_qkv, np.float32)
    w_proj = np.asarray(w_proj, np.float32)
    b_proj = np.asarray(b_proj, np.float32)

    if "nc" not in _CACHE:
        _CACHE["nc"] = _build_nc()
    nc = _CACHE["nc"]

    in_maps = [_core_inputs(x, w_qkv, w_proj, b_proj, c) for c in range(8)]
    res = run_bass_kernel_spmd(
        nc, in_maps, list(range(8)),
        trace=_trace, **(_trace_kwargs or {}),
    )
    out = np.concatenate([res.results[c]["y"] for c in range(8)], axis=2)
    if _trace:
        return out, res
    return out



# revision 8
# speedup vs baseline: 1.1023x; 1.1023x over previous
"""GroupedAttention Trainium2 kernel (8 NeuronCores, SPMD, no collectives).

Problem: x[2,2048,1024] -> grouped qkv (G=8 block-diag) -> 16-head attention
-> grouped proj (G=8 block-diag) + bias.

Sharding: core c owns heads (2c, 2c+1) and proj group c. The proj group c
consumes exactly the attention outputs of heads 2c/2c+1 and produces output
channels [128c, 128c+128) -- each core computes an independent channel
slice of the final output; outputs are concatenated on the host.

Engine plan (per core):
- PE: q/k/v projections (f32r / fp16), scores S=K^T.Q (f32r, 512-col moving),
  AV with stationary P-tile [m, n-block] and moving V|ones [m, 65] fp16
  accumulating out[n, 65] over m-tiles (the 65th column collects the softmax
  denominator), proj (fp16) with bias folded in via a K=1 ones-row matmul.
- ACT: part of the exp tiles (table Exp, scale folded in), psum->sbuf copies.
- DVE: the other exp tiles via Schraudolph bit-trick exp
  (fp16 = bitcast(int16(x*SCALE*log2e*1024 + 15315.26))), v copies,
  per-partition softmax normalization.
- SP (sync): all DMA issue - input loads, XBAR dma-transposes of the
  normalized attention tiles [n,128]->[128,n], output stores.
"""

import numpy as np
from contextlib import ExitStack

import concourse.bass as bass
import concourse.tile as tile
from concourse import bacc, mybir
from concourse.bass_utils import run_bass_kernel_spmd

F32 = mybir.dt.float32
F32R = mybir.dt.float32r
F16 = mybir.dt.float16
I16 = mybir.dt.int16
EXP = mybir.ActivationFunctionType.Exp
COPY = mybir.ActivationFunctionType.Copy

B = 2
N = 2048
C = 1024
H = 16
G = 8
D = 64          # head dim
BN = B * N      # 4096
W = 512         # attention n-window per round
NB = N // W     # rounds per batch = 4
MT = N // 128   # m-tiles per batch = 16
SCALE = D ** -0.5
LOG2E = 1.4426950408889634
# Schraudolph fp16 exp: fp16 = bitcast(int16(x*SCL1 + SCL2))
SCL1 = SCALE * LOG2E * 1024.0
SCL2 = 15315.26
# of the 16 m-tiles per window, this many run exp on ACT (rest on DVE)
ACT_TILES = 16

_CACHE = {}


def _r(ap):
    return ap if ap.dtype == F32R else ap.bitcast(F32R)


def _build_nc():
    nc = bacc.Bacc("TRN2", target_bir_lowering=False, debug=False, num_devices=8)

    xq = nc.dram_tensor("xq", [128, BN], F32, kind="ExternalInput").ap()
    xk = nc.dram_tensor("xk", [128, BN], F32, kind="ExternalInput").ap()
    xv = nc.dram_tensor("xv", [128, BN], F16, kind="ExternalInput").ap()
    wq = nc.dram_tensor("wq", [128, 128], F32, kind="ExternalInput").ap()
    wk = nc.dram_tensor("wk", [128, 128], F32, kind="ExternalInput").ap()
    wv = nc.dram_tensor("wv", [128, 128], F16, kind="ExternalInput").ap()
    wp = nc.dram_tensor("wp", [128, 128], F16, kind="ExternalInput").ap()
    brow = nc.dram_tensor("brow", [1, 128], F16, kind="ExternalInput").ap()
    y = nc.dram_tensor("y", [B, N, 128], F32, kind="ExternalOutput").ap()

    with ExitStack() as ctx:
        tc = ctx.enter_context(tile.TileContext(nc))
        nc_ = tc.nc

        persist = ctx.enter_context(tc.tile_pool(name="persist", bufs=1))

        # ---- weights / constants ----
        wq_t = persist.tile([128, 128], F32R, tag="wq")
        nc_.gpsimd.dma_start(out=wq_t, in_=wq)
        wk_t = persist.tile([128, 128], F32R, tag="wk")
        nc_.gpsimd.dma_start(out=wk_t, in_=wk)
        wv_t = persist.tile([128, 128], F16, tag="wv")
        nc_.sync.dma_start(out=wv_t, in_=wv)
        wp_t = persist.tile([128, 128], F16, tag="wp")
        nc_.sync.dma_start(out=wp_t, in_=wp)
        brow_t = persist.tile([1, 128], F16, tag="brow")
        nc_.sync.dma_start(out=brow_t, in_=brow)
        ones1 = persist.tile([1, 128], F16, tag="ones1")
        nc_.gpsimd.memset(ones1, 1.0)
        onesm = persist.tile([128, 1], F16, tag="onesm")
        nc_.gpsimd.memset(onesm, 1.0)

        # ---- x slices (channel-major) ----
        xq_t = persist.tile([128, BN], F32R, tag="xq")
        xk_t = persist.tile([128, BN], F32R, tag="xk")
        xv_t = persist.tile([128, BN], F16, tag="xv")

        # persistent activations
        qT = [persist.tile([128, N], F32R, tag=f"qT{b}", name=f"qT{b}")
              for b in range(B)]   # rows 0:64 h0, 64:128 h1 (pre-scaled? no)
        kT = [persist.tile([128, N], F32R, tag=f"kT{b}", name=f"kT{b}")
              for b in range(B)]
        # vaug[b]: [128(m), MT, 2(h), 64] fp16
        vaug = [persist.tile([128, MT, 2, 64], F16, tag=f"vaug{b}", name=f"vaug{b}")
                for b in range(B)]
        # transposed, normalized attention outputs per b: [128(2h*d), N] fp16
        attT = [persist.tile([128, N], F16, tag=f"attT{b}", name=f"attT{b}")
                for b in range(B)]

        # ---- load x: per-batch chunks ordered k, v, q so windows start early
        for b in range(B):
            s = slice(b * N, (b + 1) * N)
            for i in range(2):
                ss = slice(b * N + i * 1024, b * N + (i + 1) * 1024)
                nc_.gpsimd.dma_start(out=xk_t[:, ss], in_=xk[:, ss])
            nc_.sync.dma_start(out=xv_t[:, s], in_=xv[:, s])
            for i in range(2):
                ss = slice(b * N + i * 1024, b * N + (i + 1) * 1024)
                nc_.gpsimd.dma_start(out=xq_t[:, ss], in_=xq[:, ss])

        # ---- phase 1: kT / vaug / qT per batch ----
        with tc.tile_pool(name="ph1", bufs=3, space="PSUM") as ph1:
            for b in range(B):
                for i in range(4):
                    sl = slice(i * 512, (i + 1) * 512)
                    s = slice(b * N + i * 512, b * N + (i + 1) * 512)
                    pk = ph1.tile([128, 512], F32, tag="qk")
                    nc_.tensor.matmul(pk, _r(wk_t), _r(xk_t[:, s]), start=True, stop=True)
                    nc_.scalar.activation(out=kT[b][:, sl], in_=pk, func=COPY)
                for mt in range(MT):
                    g = b * MT + mt
                    pv = ph1.tile([128, 128], F32, tag="v")
                    nc_.tensor.matmul(
                        pv, xv_t[:, g * 128:(g + 1) * 128], wv_t,
                        start=True, stop=True,
                    )
                    # both heads' v [128, (2,64)] -> vaug[b][:, mt]
                    nc_.vector.tensor_copy(out=vaug[b][:, mt], in_=pv)
                for i in range(4):
                    sl = slice(i * 512, (i + 1) * 512)
                    s = slice(b * N + i * 512, b * N + (i + 1) * 512)
                    pq = ph1.tile([128, 512], F32, tag="qk")
                    nc_.tensor.matmul(pq, _r(wq_t), _r(xq_t[:, s]), start=True, stop=True)
                    nc_.scalar.activation(out=qT[b][:, sl], in_=pq, func=COPY)

        # ---- phase 2: attention windows + pipelined proj ----
        with tc.tile_pool(name="stp", bufs=2, space="PSUM") as stp, \
             tc.tile_pool(name="avp", bufs=2, space="PSUM") as avp, \
             tc.tile_pool(name="prj", bufs=1, space="PSUM") as prj, \
             tc.tile_pool(name="ptp", bufs=3) as ptp, \
             tc.tile_pool(name="nrm", bufs=2) as nrm, \
             tc.tile_pool(name="atn", bufs=2) as atn, \
             tc.tile_pool(name="outp", bufs=2) as outp:

            def emit_proj(b, nb):
                # proj for window (b, nb): out[n,cout] = attT^T @ wp + bias
                n0 = nb * W
                pp = prj.tile([128, 4, 128], F32, tag="pp", name=f"pp{b}{nb}")
                for j in range(4):
                    nt = slice(n0 + j * 128, n0 + (j + 1) * 128)
                    nc_.tensor.matmul(pp[:, j, :], attT[b][:, nt], wp_t,
                                      start=True, stop=False)
                    nc_.tensor.matmul(pp[:, j, :], ones1, brow_t,
                                      start=False, stop=True)
                ot = outp.tile([128, 4, 128], F32, tag="ot", name=f"ot{b}{nb}")
                nc_.scalar.activation(out=ot, in_=pp, func=COPY)
                # y[b, n0+j*128+p, c] <- ot[p, j, c]
                yap = y[b, n0:n0 + W, :].rearrange("(j p) c -> p j c", p=128)
                nc_.sync.dma_start(out=yap, in_=ot)

            pending_proj = None
            for b in range(B):
                for nb in range(NB):
                    n0 = nb * W
                    # av: [128(n), 2(h), 4(j), 64(d)], z: [128, 2, 4, 1];
                    # both accumulated over mt
                    av = avp.tile([128, 2, 4, 64], F32, tag="av",
                                  name=f"av{b}{nb}")
                    zz = avp.tile([128, 2, 4, 1], F32, tag="z", bufs=1,
                                  name=f"z{b}{nb}")

                    def emit_av(mt, pt):
                        for h in range(2):
                            for j in range(4):
                                first = (mt == 0 and h == 0 and j == 0)
                                nc_.tensor.matmul(
                                    av[:, h, j, :],
                                    pt[:, h, j, :],
                                    vaug[b][:, mt, h, :],
                                    start=first, stop=(mt == MT - 1),
                                    skip_group_check=True,
                                )
                                nc_.tensor.matmul(
                                    zz[:, h, j, :],
                                    pt[:, h, j, :],
                                    onesm,
                                    start=first, stop=(mt == MT - 1),
                                    skip_group_check=True,
                                )

                    # software pipeline: PE does scores(mt) then AV(mt-1)
                    # while ACT/DVE run exp(mt).
                    prev = None
                    for mt in range(MT):
                        m0 = mt * 128
                        st = stp.tile([128, 2, 512], F32, tag="st")
                        for h in range(2):
                            hs = slice(h * 64, (h + 1) * 64)
                            nc_.tensor.matmul(
                                st[:, h, :],
                                _r(kT[b][hs, m0:m0 + 128]),
                                _r(qT[b][hs, n0:n0 + W]),
                                start=True, stop=True,
                            )
                        if prev is not None:
                            emit_av(*prev)
                        elif pending_proj is not None:
                            emit_proj(*pending_proj)
                            pending_proj = None
                        pt = ptp.tile([128, 2, 4, 128], F16, tag="pt")
                        if mt < ACT_TILES:
                            nc_.scalar.activation(out=pt, in_=st, func=EXP,
                                                  scale=SCALE)
                        else:
                            nc_.vector.tensor_scalar(
                                out=pt.bitcast(I16),
                                in0=st,
                                scalar1=SCL1,
                                scalar2=SCL2,
                                op0=mybir.AluOpType.mult,
                                op1=mybir.AluOpType.add,
                            )
                        prev = (mt, pt)
                    emit_av(*prev)

                    # normalize: av[:, h, j, :] * (1 / z[:, h, j])
                    attn_n = atn.tile([128, 4, 2, 64], F16, tag="attn",
                                      name=f"attn{b}{nb}")
                    zinv = nrm.tile([128, 2, 4], F32, tag="zinv")
                    nc_.vector.reciprocal_approx_fast(
                        out=zinv, in_=zz[:, :, :, 0])
                    for h in range(2):
                        for j in range(4):
                            nc_.vector.tensor_scalar(
                                out=attn_n[:, j, h, :],
                                in0=av[:, h, j, :],
                                scalar1=zinv[:, h, j:j + 1],
                                scalar2=None,
                                op0=mybir.AluOpType.mult,
                            )
                    # transpose [n,128] -> [128,n] per 128-block via DMA XBAR
                    for j in range(4):
                        nc_.sync.dma_start_transpose(
                            out=attT[b][:, n0 + j * 128:n0 + (j + 1) * 128],
                            in_=attn_n[:, j, :, :],
                        )
                    pending_proj = (b, nb)
            emit_proj(*pending_proj)

    nc.finalize()
    return nc


def _core_inputs(x, w_qkv, w_proj, b_proj, c):
    h0 = 2 * c
    gq, oq = divmod(64 * h0, 384)
    gk, ok = divmod(C + 64 * h0, 384)
    gv, ov = divmod(2 * C + 64 * h0, 384)

    def xsl(g, dt=np.float32):
        # [B,N,128] slice -> channel-major [128, B*N]
        return np.ascontiguousarray(
            x[:, :, 128 * g:128 * (g + 1)].reshape(BN, 128).T
        ).astype(dt)

    return {
        "xq": xsl(gq),
        "xk": xsl(gk),
        "xv": xsl(gv, np.float16),
        "wq": np.ascontiguousarray(w_qkv[gq][:, oq:oq + 128]),
        "wk": np.ascontiguousarray(w_qkv[gk][:, ok:ok + 128]),
        "wv": np.ascontiguousarray(w_qkv[gv][:, ov:ov + 128]).astype(np.float16),
        "wp": np.ascontiguousarray(w_proj[c]).astype(np.float16),
        "brow": b_proj[128 * c:128 * (c + 1)].reshape(1, 128).astype(np.float16),
    }


def kernel(x, w_qkv, w_proj, b_proj, _trace=False, _trace_kwargs=None):
    x = np.asarray(x, np.float32)
    w_qkv = np.asarray(w_qkv, np.float32)
    w_proj = np.asarray(w_proj, np.float32)
    b_proj = np.asarray(b_proj, np.float32)

    if "nc" not in _CACHE:
        _CACHE["nc"] = _build_nc()
    nc = _CACHE["nc"]

    in_maps = [_core_inputs(x, w_qkv, w_proj, b_proj, c) for c in range(8)]
    res = run_bass_kernel_spmd(
        nc, in_maps, list(range(8)),
        trace=_trace, **(_trace_kwargs or {}),
    )
    out = np.concatenate([res.results[c]["y"] for c in range(8)], axis=2)
    if _trace:
        return out, res
    return out


# revision 12
# speedup vs baseline: 1.2915x; 1.1716x over previous
"""GroupedAttention Trainium2 kernel (8 NeuronCores, SPMD, no collectives).

Problem: x[2,2048,1024] -> grouped qkv (G=8 block-diag) -> 16-head attention
-> grouped proj (G=8 block-diag) + bias.

Sharding: core c owns heads (2c, 2c+1) and proj group c. The proj group c
consumes exactly the attention outputs of heads 2c/2c+1 and produces output
channels [128c, 128c+128) -- each core computes an independent channel
slice of the final output; outputs are concatenated on the host.

Engine plan (per core):
- PE: q/k/v projections (f32r / fp16), scores S=K^T.Q (f32r, 512-col moving),
  AV with stationary P-tile [m, n-block] and moving V|ones [m, 65] fp16
  accumulating out[n, 65] over m-tiles (the 65th column collects the softmax
  denominator), proj (fp16) with bias folded in via a K=1 ones-row matmul.
- ACT: part of the exp tiles (table Exp, scale folded in), psum->sbuf copies.
- DVE: the other exp tiles via Schraudolph bit-trick exp
  (fp16 = bitcast(int16(x*SCALE*log2e*1024 + 15315.26))), v copies,
  per-partition softmax normalization.
- SP (sync): all DMA issue - input loads, XBAR dma-transposes of the
  normalized attention tiles [n,128]->[128,n], output stores.
"""

import numpy as np
from contextlib import ExitStack

import concourse.bass as bass
import concourse.tile as tile
from concourse import bacc, mybir
from concourse.bass_utils import run_bass_kernel_spmd

F32 = mybir.dt.float32
F32R = mybir.dt.float32r
F16 = mybir.dt.float16
I16 = mybir.dt.int16
EXP = mybir.ActivationFunctionType.Exp
COPY = mybir.ActivationFunctionType.Copy

B = 2
N = 2048
C = 1024
H = 16
G = 8
D = 64          # head dim
BN = B * N      # 4096
W = 512         # attention n-window per round
NB = N // W     # rounds per batch = 4
MT = N // 128   # m-tiles per batch = 16
SCALE = D ** -0.5
LOG2E = 1.4426950408889634
# Schraudolph fp16 exp: fp16 = bitcast(int16(x*SCL1 + SCL2))
SCL1 = SCALE * LOG2E * 1024.0
SCL2 = 15315.26
# of the 16 m-tiles per window, this many run exp on ACT (rest on DVE)
ACT_TILES = 16

_CACHE = {}


def _r(ap):
    return ap if ap.dtype == F32R else ap.bitcast(F32R)


def _build_nc():
    nc = bacc.Bacc("TRN2", target_bir_lowering=False, debug=False, num_devices=8)

    xq = nc.dram_tensor("xq", [128, BN], F32, kind="ExternalInput").ap()
    xk = nc.dram_tensor("xk", [128, BN], F32, kind="ExternalInput").ap()
    xv = nc.dram_tensor("xv", [128, BN], F16, kind="ExternalInput").ap()
    wq = nc.dram_tensor("wq", [128, 128], F32, kind="ExternalInput").ap()
    wk = nc.dram_tensor("wk", [128, 128], F32, kind="ExternalInput").ap()
    wv = nc.dram_tensor("wv", [128, 128], F16, kind="ExternalInput").ap()
    wp = nc.dram_tensor("wp", [128, 128], F16, kind="ExternalInput").ap()
    brow = nc.dram_tensor("brow", [1, 128], F16, kind="ExternalInput").ap()
    y = nc.dram_tensor("y", [B, N, 128], F32, kind="ExternalOutput").ap()

    with ExitStack() as ctx:
        tc = ctx.enter_context(tile.TileContext(nc))
        nc_ = tc.nc

        persist = ctx.enter_context(tc.tile_pool(name="persist", bufs=1))

        # ---- weights / constants ----
        wq_t = persist.tile([128, 128], F32R, tag="wq")
        nc_.gpsimd.dma_start(out=wq_t, in_=wq)
        wk_t = persist.tile([128, 128], F32R, tag="wk")
        nc_.gpsimd.dma_start(out=wk_t, in_=wk)
        wv_t = persist.tile([128, 128], F16, tag="wv")
        nc_.sync.dma_start(out=wv_t, in_=wv)
        wp_t = persist.tile([128, 128], F16, tag="wp")
        nc_.sync.dma_start(out=wp_t, in_=wp)
        brow_t = persist.tile([1, 128], F16, tag="brow")
        nc_.sync.dma_start(out=brow_t, in_=brow)
        ones1 = persist.tile([1, 128], F16, tag="ones1")
        nc_.gpsimd.memset(ones1, 1.0)
        onesm = persist.tile([128, 1], F16, tag="onesm")
        nc_.gpsimd.memset(onesm, 1.0)
        ebias = persist.tile([128, 1], F32, tag="ebias")
        nc_.gpsimd.memset(ebias, EXP_BIAS)

        # ---- x slices (channel-major) ----
        xq_t = persist.tile([128, BN], F32R, tag="xq")
        xk_t = persist.tile([128, BN], F32R, tag="xk")
        xv_t = persist.tile([128, BN], F16, tag="xv")

        # persistent activations
        qT = [persist.tile([128, N], F32R, tag=f"qT{b}", name=f"qT{b}")
              for b in range(B)]   # rows 0:64 h0, 64:128 h1 (pre-scaled? no)
        kT = [persist.tile([128, N], F32R, tag=f"kT{b}", name=f"kT{b}")
              for b in range(B)]
        # vaug[b]: [128(m), MT, 2(h), 64] fp16
        vaug = [persist.tile([128, MT, 2, 64], F16, tag=f"vaug{b}", name=f"vaug{b}")
                for b in range(B)]
        # transposed, normalized attention outputs per b: [128(2h*d), N] fp16
        attT = [persist.tile([128, N], F16, tag=f"attT{b}", name=f"attT{b}")
                for b in range(B)]

        # ---- load x: per-batch chunks ordered k, v, q so windows start early
        for b in range(B):
            s = slice(b * N, (b + 1) * N)
            for i in range(2):
                ss = slice(b * N + i * 1024, b * N + (i + 1) * 1024)
                nc_.gpsimd.dma_start(out=xk_t[:, ss], in_=xk[:, ss])
            nc_.sync.dma_start(out=xv_t[:, s], in_=xv[:, s])
            for i in range(2):
                ss = slice(b * N + i * 1024, b * N + (i + 1) * 1024)
                nc_.gpsimd.dma_start(out=xq_t[:, ss], in_=xq[:, ss])

        # ---- phase 1: kT / vaug / qT per batch ----
        with tc.tile_pool(name="ph1", bufs=3, space="PSUM") as ph1:
            for b in range(B):
                for i in range(4):
                    sl = slice(i * 512, (i + 1) * 512)
                    s = slice(b * N + i * 512, b * N + (i + 1) * 512)
                    pk = ph1.tile([128, 512], F32, tag="qk")
                    nc_.tensor.matmul(pk, _r(wk_t), _r(xk_t[:, s]), start=True, stop=True)
                    nc_.scalar.activation(out=kT[b][:, sl], in_=pk, func=COPY)
                for mt in range(MT):
                    g = b * MT + mt
                    pv = ph1.tile([128, 128], F32, tag="v")
                    nc_.tensor.matmul(
                        pv, xv_t[:, g * 128:(g + 1) * 128], wv_t,
                        start=True, stop=True,
                    )
                    # both heads' v [128, (2,64)] -> vaug[b][:, mt]
                    nc_.vector.tensor_copy(out=vaug[b][:, mt], in_=pv)
                for i in range(4):
                    sl = slice(i * 512, (i + 1) * 512)
                    s = slice(b * N + i * 512, b * N + (i + 1) * 512)
                    pq = ph1.tile([128, 512], F32, tag="qk")
                    nc_.tensor.matmul(pq, _r(wq_t), _r(xq_t[:, s]), start=True, stop=True)
                    nc_.scalar.activation(out=qT[b][:, sl], in_=pq, func=COPY)

        # ---- phase 2: flat software-pipelined attention + proj ----
        with tc.tile_pool(name="stp", bufs=2, space="PSUM") as stp, \
             tc.tile_pool(name="avp", bufs=2, space="PSUM") as avp, \
             tc.tile_pool(name="prj", bufs=1, space="PSUM") as prj, \
             tc.tile_pool(name="ptp", bufs=4) as ptp, \
             tc.tile_pool(name="nrm", bufs=2) as nrm, \
             tc.tile_pool(name="atn", bufs=2) as atn, \
             tc.tile_pool(name="outp", bufs=2) as outp:

            LAG = 2
            seq = [(b, nb, mt) for b in range(B) for nb in range(NB)
                   for mt in range(MT)]
            act_set = {mt for mt in range(MT)
                       if (mt + 1) * ACT_TILES // MT > mt * ACT_TILES // MT}
            avs = {}
            pts = {}

            def emit_proj(w):
                # out[n, cout] = attT^T @ wp + ones^T @ bias
                b, nb = divmod(w, NB)
                n0 = nb * W
                pp = prj.tile([128, 4, 128], F32, tag="pp", name=f"pp{w}")
                for j in range(4):
                    nt = slice(n0 + j * 128, n0 + (j + 1) * 128)
                    nc_.tensor.matmul(pp[:, j, :], attT[b][:, nt], wp_t,
                                      start=(j == 0), stop=False,
                                      skip_group_check=True)
                    nc_.tensor.matmul(pp[:, j, :], ones1, brow_t,
                                      start=False, stop=(j == 3),
                                      skip_group_check=True)
                ot = outp.tile([128, 4, 128], F32, tag="ot", name=f"ot{w}")
                nc_.scalar.activation(out=ot, in_=pp, func=COPY)
                yap = y[b, n0:n0 + W, :].rearrange("(j p) c -> p j c", p=128)
                nc_.sync.dma_start(out=yap, in_=ot)

            def emit_post(w, av, zz):
                # normalize av by 1/z (per-partition), then DMA-transpose the
                # [n, (h d)] fp16 tiles into attT[b]
                b, nb = divmod(w, NB)
                n0 = nb * W
                zinv = nrm.tile([128, 2, 4, 1], F32, tag="zinv")
                nc_.vector.reciprocal_approx_fast(
                    out=zinv.rearrange("p h j o -> p (h j o)"),
                    in_=zz.rearrange("p h j o -> p (h j o)"))
                attn_n = atn.tile([128, 4, 2, 64], F16, tag="attn",
                                  name=f"attn{w}")
                nc_.vector.tensor_tensor(
                    out=attn_n.rearrange("p j h d -> p h j d"),
                    in0=av,
                    in1=zinv.broadcast_to([128, 2, 4, 64]),
                    op=mybir.AluOpType.mult,
                )
                for j in range(4):
                    nc_.sync.dma_start_transpose(
                        out=attT[b][:, n0 + j * 128:n0 + (j + 1) * 128],
                        in_=attn_n[:, j, :, :],
                    )

            pending_proj = None
            for k in range(len(seq) + LAG):
                if k < len(seq):
                    b, nb, mt = seq[k]
                    w = k // MT
                    if mt == 0:
                        avs[w] = (
                            avp.tile([128, 2, 4, 64], F32, tag="av",
                                     name=f"av{w}"),
                            avp.tile([128, 2, 4, 1], F32, tag="z", bufs=1,
                                     name=f"z{w}"),
                        )
                    n0 = nb * W
                    m0 = mt * 128
                    st = stp.tile([128, 2, 512], F32, tag="st")
                    for h in range(2):
                        hs = slice(h * 64, (h + 1) * 64)
                        nc_.tensor.matmul(
                            st[:, h, :],
                            _r(kT[b][hs, m0:m0 + 128]),
                            _r(qT[b][hs, n0:n0 + W]),
                            start=True, stop=True,
                        )
                    pt = ptp.tile([128, 2, 4, 128], F16, tag="pt")
                    if mt in act_set:
                        nc_.scalar.activation(out=pt, in_=st, func=EXP,
                                              scale=SCALE, bias=ebias)
                    else:
                        nc_.vector.tensor_scalar(
                            out=pt.bitcast(I16),
                            in0=st,
                            scalar1=SCL1,
                            scalar2=SCL2,
                            op0=mybir.AluOpType.mult,
                            op1=mybir.AluOpType.add,
                        )
                    pts[k] = pt

                kk = k - LAG
                if kk < 0:
                    continue
                b2, nb2, mt2 = seq[kk]
                w2 = kk // MT
                av, zz = avs[w2]
                pt2 = pts.pop(kk)
                for h in range(2):
                    for j in range(4):
                        first = (mt2 == 0 and h == 0 and j == 0)
                        nc_.tensor.matmul(
                            av[:, h, j, :],
                            pt2[:, h, j, :],
                            vaug[b2][:, mt2, h, :],
                            start=first, stop=(mt2 == MT - 1),
                            skip_group_check=True,
                        )
                        nc_.tensor.matmul(
                            zz[:, h, j, :],
                            pt2[:, h, j, :],
                            onesm,
                            start=first, stop=(mt2 == MT - 1),
                            skip_group_check=True,
                        )
                if mt2 == 6 and pending_proj is not None:
                    emit_proj(pending_proj)
                    pending_proj = None
                if mt2 == MT - 1:
                    emit_post(w2, av, zz)
                    del avs[w2]
                    pending_proj = w2
            emit_proj(pending_proj)

    nc.finalize()
    return nc


def _core_inputs(x, w_qkv, w_proj, b_proj, c):
    h0 = 2 * c
    gq, oq = divmod(64 * h0, 384)
    gk, ok = divmod(C + 64 * h0, 384)
    gv, ov = divmod(2 * C + 64 * h0, 384)

    def xsl(g, dt=np.float32):
        # [B,N,128] slice -> channel-major [128, B*N]
        return np.ascontiguousarray(
            x[:, :, 128 * g:128 * (g + 1)].reshape(BN, 128).T
        ).astype(dt)

    return {
        "xq": xsl(gq),
        "xk": xsl(gk),
        "xv": xsl(gv, np.float16),
        "wq": np.ascontiguousarray(w_qkv[gq][:, oq:oq + 128]),
        "wk": np.ascontiguousarray(w_qkv[gk][:, ok:ok + 128]),
        "wv": np.ascontiguousarray(w_qkv[gv][:, ov:ov + 128]).astype(np.float16),
        "wp": np.ascontiguousarray(w_proj[c]).astype(np.float16),
        "brow": b_proj[128 * c:128 * (c + 1)].reshape(1, 128).astype(np.float16),
    }


def kernel(x, w_qkv, w_proj, b_proj, _trace=False, _trace_kwargs=None):
    x = np.asarray(x, np.float32)
    w_qkv = np.asarray(w_qkv, np.float32)
    w_proj = np.asarray(w_proj, np.float32)
    b_proj = np.asarray(b_proj, np.float32)

    if "nc" not in _CACHE:
        _CACHE["nc"] = _build_nc()
    nc = _CACHE["nc"]

    in_maps = [_core_inputs(x, w_qkv, w_proj, b_proj, c) for c in range(8)]
    res = run_bass_kernel_spmd(
        nc, in_maps, list(range(8)),
        trace=_trace, **(_trace_kwargs or {}),
    )
    out = np.concatenate([res.results[c]["y"] for c in range(8)], axis=2)
    if _trace:
        return out, res
    return out
